# revision 21
# baseline (speedup 1.0000x reference)
import sys

sys.path.insert(0, "/opt/trn_rl_repo")

import numpy as np
import ml_dtypes

import concourse.bacc as bacc
import concourse.bass as bass
import concourse.mybir as mybir
import concourse.tile as tile
from concourse.bass_utils import run_bass_kernel_spmd

F32 = mybir.dt.float32
F32R = mybir.dt.float32r
BF16 = mybir.dt.bfloat16
AF = mybir.ActivationFunctionType
ALU = mybir.AluOpType
AX = mybir.AxisListType

# Problem constants (hardcoded per harness contract).
B, C, H, W = 4, 64, 128, 128
NT = 9          # 3x3 taps
NFF = 4         # factor*factor subpixels
NCORES = 8
HL = H // 2     # 64 coarse rows per core
NYB = 4         # y-blocks
YB = HL // NYB  # 16 coarse rows per block
G = NFF * C * YB  # 4096 elements per tap-slice per partition

_cached = {}


def ap_of(t, off, dims):
    base = t[:]
    return bass.AP(base.tensor, base.offset + off, dims)


def build_nc():
    nc = bacc.Bacc("TRN2", target_bir_lowering=False, debug=False, num_devices=NCORES)

    hpa_d = nc.dram_tensor("hpa", [128, 66 * 128], BF16, kind="ExternalInput")
    hpb_d = nc.dram_tensor("hpb", [64, 66 * 128], BF16, kind="ExternalInput")
    ht_d = nc.dram_tensor("ht", [128, 4 * 9 * 64 * 16], BF16, kind="ExternalInput")
    w1a_d = nc.dram_tensor("w1a", [128, 3 * 128], BF16, kind="ExternalInput")
    w1b_d = nc.dram_tensor("w1b", [64, 3 * 128], BF16, kind="ExternalInput")
    b1_d = nc.dram_tensor("b1c", [128, 1], F32, kind="ExternalInput")
    w2t_d = nc.dram_tensor("w2t", [128, 36], BF16, kind="ExternalInput")
    b2_d = nc.dram_tensor("b2c", [36, 1], F32, kind="ExternalInput")
    idb_d = nc.dram_tensor("idb", [128, 128], BF16, kind="ExternalInput")
    out_d = nc.dram_tensor("out", [128, 64 * 256], F32, kind="ExternalOutput")

    with tile.TileContext(nc) as tc:
        with (
            tc.tile_pool(name="const", bufs=1) as cpool,
            tc.tile_pool(name="rin", bufs=2) as rin,
            tc.tile_pool(name="mchunk", bufs=3) as mpool,
            tc.tile_pool(name="rmask", bufs=2) as rmask,
            tc.tile_pool(name="work", bufs=1) as work,
            tc.tile_pool(name="racc", bufs=2) as racc,
            tc.tile_pool(name="orow", bufs=3) as opool,
            tc.tile_pool(name="ps1", bufs=2, space=bass.MemorySpace.PSUM) as pp1,
            tc.tile_pool(name="ps2", bufs=2, space=bass.MemorySpace.PSUM) as pp2,
            tc.tile_pool(name="pst", bufs=2, space=bass.MemorySpace.PSUM) as ppt,
            tc.tile_pool(name="pso", bufs=2, space=bass.MemorySpace.PSUM) as ppo,
        ):
            # ---- constants ----
            w1a = cpool.tile([128, 3 * 128], BF16)
            w1b = cpool.tile([64, 3 * 128], BF16)
            b1 = cpool.tile([128, 1], F32)
            w2t = cpool.tile([128, 36], BF16)
            b2 = cpool.tile([36, 1], F32)
            idb = cpool.tile([128, 128], BF16)
            nc.sync.dma_start(w1a[:], w1a_d[:])
            nc.sync.dma_start(w1b[:], w1b_d[:])
            nc.sync.dma_start(b1[:], b1_d[:])
            nc.sync.dma_start(w2t[:], w2t_d[:])
            nc.sync.dma_start(b2[:], b2_d[:])
            nc.sync.dma_start(idb[:], idb_d[:])

            BLOCKS = [(0, 8), (8, 16), (24, 16), (40, 16), (56, 8)]
            for r0, nr in BLOCKS:
                nch = nr // 4           # conv chunks (4 rows each)
                gb = NFF * C * nr       # elements per tap-slice per partition
                fcr = 64 * nr           # ff/c stride in prod; per-tap htb size
                hto = 9 * 64 * r0       # cumulative ht offset (uniform row cost)
                hpab = rin.tile([128, 18 * 128], BF16, tag="hpab")
                hpbb = rin.tile([64, 18 * 128], BF16, tag="hpbb")
                # htb layout: [x, (t 9, c 64, yl nr)]
                htb = rin.tile([128, 9 * 64 * 16], BF16, tag="htb")
                nc.sync.dma_start(hpab[:, 0:(nr + 2) * 128],
                                  hpa_d[:, r0 * 128:(r0 + nr + 2) * 128])
                nc.sync.dma_start(hpbb[:, 0:(nr + 2) * 128],
                                  hpb_d[:, r0 * 128:(r0 + nr + 2) * 128])
                nc.sync.dma_start(htb[:, 0:9 * fcr],
                                  ht_d[:, hto:hto + 9 * fcr])

                # ---- conv1 -> relu -> conv2 -> exp ----
                eb = rmask.tile([36, 4 * 512], BF16, tag="eb")
                for ic in range(nch):
                    ps1 = pp1.tile([128, 512], F32)
                    for dy in range(3):
                        rhs = ap_of(hpab, (4 * ic + dy) * 128,
                                    [[18 * 128, 128], [1, 512]])
                        nc.tensor.matmul(ps1[:],
                                         w1a[:, dy * 128:(dy + 1) * 128],
                                         rhs, start=(dy == 0), stop=False)
                    for dy in range(3):
                        rhs = ap_of(hpbb, (4 * ic + dy) * 128,
                                    [[18 * 128, 64], [1, 512]])
                        nc.tensor.matmul(ps1[:],
                                         w1b[:, dy * 128:(dy + 1) * 128],
                                         rhs, start=False, stop=(dy == 2))
                    m = mpool.tile([128, 512], BF16)
                    nc.scalar.activation(m[:], ps1[:], AF.Relu, bias=b1[:], scale=1.0)
                    ps2 = pp2.tile([36, 512], F32)
                    nc.tensor.matmul(ps2[:], w2t[:], m[:])
                    nc.scalar.activation(eb[:, ic * 512:(ic + 1) * 512],
                                         ps2[:], AF.Exp, bias=b2[:], scale=1.0)

                # ---- mask transpose + reshuffle; Z and 1/Z ----
                # nmb layout: [x, (ff 4, t 9, yl nr)]; zt/rz: [x, (ff 4, yl nr)]
                zt = rmask.tile([128, 64], F32, tag="zt")
                rz = rmask.tile([128, 64], F32, tag="rz")
                nmb = rmask.tile([128, NFF * NT * YB], BF16, tag="nmb")
                for j in range(nch):
                    pst = ppt.tile([128, 4 * 36], BF16)
                    for r in range(4):
                        yl = 4 * j + r
                        nc.tensor.transpose(pst[:, r * 36:(r + 1) * 36],
                                            eb[:, yl * 128:(yl + 1) * 128],
                                            idb[0:36, 0:36])
                    nm_out = ap_of(nmb, 4 * j,
                                   [[NFF * NT * YB, 128], [NT * nr, 4], [nr, 9], [1, 4]])
                    nm_in = ap_of(pst, 0, [[4 * 36, 128], [9, 4], [1, 9], [36, 4]])
                    nc.scalar.copy(nm_out, nm_in)
                nc.vector.tensor_reduce(
                    ap_of(zt, 0, [[64, 128], [nr, 4], [1, nr]]),
                    ap_of(nmb, 0, [[NFF * NT * YB, 128], [9 * nr, 4], [1, nr], [nr, 9]]),
                    AX.X, ALU.add)
                nc.vector.reciprocal(rz[:, 0:4 * nr], zt[:, 0:4 * nr])

                # normalized mask: nm2 = nm * (1/Z)
                nm2 = rmask.tile([128, NFF * NT * YB], BF16, tag="nm2")
                nc.vector.tensor_tensor(
                    ap_of(nm2, 0, [[NFF * NT * YB, 128], [9 * nr, 4], [nr, 9], [1, nr]]),
                    ap_of(nmb, 0, [[NFF * NT * YB, 128], [9 * nr, 4], [nr, 9], [1, nr]]),
                    ap_of(rz, 0, [[64, 128], [nr, 4], [0, 9], [1, nr]]),
                    ALU.mult)

                # ---- weighted tap sum (TT 2x bf16, all on DVE) ----
                # prod layout: [x, (t 9, ff 4, c 64, yl nr)]
                prod = work.tile([128, NT * G], BF16, tag="prod")
                tA = work.tile([128, 4 * G], BF16, tag="tA")
                acc = racc.tile([128, G], BF16, tag="acc")

                def pr(off, n):
                    return ap_of(prod, off * gb, [[NT * G, 128], [1, n * gb]])

                def ta(off, n):
                    return ap_of(tA, off * gb, [[4 * G, 128], [1, n * gb]])

                for t in range(9):
                    in0 = ap_of(htb, t * fcr,
                                [[9 * 1024 * 16 // 16, 128], [0, 4], [nr, 64], [1, nr]])
                    in1 = ap_of(nm2, t * nr,
                                [[NFF * NT * YB, 128], [9 * nr, 4], [0, 64], [1, nr]])
                    po = ap_of(prod, t * gb,
                               [[NT * G, 128], [fcr, 4], [nr, 64], [1, nr]])
                    nc.vector.tensor_tensor(po, in0, in1, ALU.mult)
                nc.vector.tensor_tensor(tA[:, 0:4 * gb], pr(0, 4), pr(4, 4), ALU.add)
                nc.vector.tensor_tensor(pr(0, 2), ta(0, 2), ta(2, 2), ALU.add)
                nc.vector.tensor_tensor(ta(0, 1), pr(0, 1), pr(1, 1), ALU.add)
                for q in range(nch):
                    nc.vector.tensor_tensor(
                        ap_of(acc, 4 * q, [[G, 128], [nr, 256], [1, 4]]),
                        ap_of(tA, 4 * q, [[4 * G, 128], [nr, 256], [1, 4]]),
                        ap_of(prod, 8 * gb + 4 * q, [[NT * G, 128], [nr, 256], [1, 4]]),
                        ALU.add)

                # ---- pixel shuffle out: transpose (x,(fy,c)) -> ((fy,c),x) ----
                for yp in range(nch):  # groups of 4 coarse rows
                    pso = ppo.tile([128, 1024], BF16)
                    for j4 in range(4):
                        yl = 4 * yp + j4
                        for fx in range(2):
                            in_ap = ap_of(acc, fx * 2 * fcr + yl,
                                          [[G, 128], [nr, 128]])
                            nc.tensor.transpose(
                                pso[:, (2 * j4 + fx) * 128:(2 * j4 + fx + 1) * 128],
                                in_ap, idb[:])
                    orow = opool.tile([128, 1024], F32)
                    co = ap_of(orow, 0, [[1024, 128], [256, 4], [1, 2], [2, 128]])
                    ci = ap_of(pso, 0, [[1024, 128], [256, 4], [128, 2], [1, 128]])
                    nc.scalar.copy(co, ci)
                    y = r0 + 4 * yp
                    nc.sync.dma_start(
                        ap_of(out_d, y * 256, [[64 * 256, 128], [1, 1024]]),
                        orow[:])

    nc.compile()
    return nc


def prep_shared(W1, b1, W2, b2):
    W1 = np.asarray(W1, np.float32)
    b1 = np.asarray(b1, np.float32)
    W2 = np.asarray(W2, np.float32).reshape(36, 128)
    b2 = np.asarray(b2, np.float32)

    w1a = np.zeros((128, 3 * 128), np.float32)
    w1b = np.zeros((64, 3 * 128), np.float32)
    for dy in range(3):
        w1a[0:64, dy * 128:(dy + 1) * 128] = W1[:, :, dy, 0].T
        w1a[64:128, dy * 128:(dy + 1) * 128] = W1[:, :, dy, 1].T
        w1b[:, dy * 128:(dy + 1) * 128] = W1[:, :, dy, 2].T

    # ffT = fx*2+fy ordering: mask channel for (ffT, t) is t*4 + (fy*2+fx)
    o_of_mp = np.array([t * 4 + 2 * (ffT % 2) + ffT // 2
                        for ffT in range(4) for t in range(9)])
    w2t = np.ascontiguousarray((0.25 * W2[o_of_mp, :]).T)
    b2c = np.ascontiguousarray((0.25 * b2[o_of_mp]).reshape(36, 1))

    return {
        "w1a": w1a.astype(ml_dtypes.bfloat16),
        "w1b": w1b.astype(ml_dtypes.bfloat16),
        "b1c": b1.reshape(128, 1).astype(np.float32),
        "w2t": w2t.astype(ml_dtypes.bfloat16), "b2c": b2c,
        "idb": np.eye(128, dtype=ml_dtypes.bfloat16),
    }


def kernel(h, W1, b1, W2, b2, _trace=False):
    h = np.asarray(h, np.float32)
    shared = prep_shared(W1, b1, W2, b2)

    hp = np.pad(h, ((0, 0), (0, 0), (1, 1), (1, 1)))  # [B, C, 130, 130]
    in_maps = []
    for core in range(NCORES):
        b, half = core // 2, core % 2
        y0 = half * HL
        win = hp[b, :, y0:y0 + 66, :]  # [64, 66, 130]
        winb = win.astype(ml_dtypes.bfloat16)
        hpa = np.empty((128, 66, 128), ml_dtypes.bfloat16)
        hpa[0:64] = winb[:, :, 0:128]
        hpa[64:128] = winb[:, :, 1:129]
        hpb = np.ascontiguousarray(winb[:, :, 2:130])
        win8 = (8.0 * win).astype(ml_dtypes.bfloat16)
        # ht: per-block segments [x, (t 9, c 64, yl nr)], blocks (0,8),(8,16)x3,(56,8)
        segs = []
        for r0, nr in [(0, 8), (8, 16), (24, 16), (40, 16), (56, 8)]:
            seg = np.empty((128, 9, 64, nr), ml_dtypes.bfloat16)
            for dy in range(3):
                for dx in range(3):
                    seg[:, dy * 3 + dx] = win8[
                        :, r0 + dy:r0 + dy + nr, dx:dx + 128].transpose(2, 0, 1)
            segs.append(seg.reshape(128, -1))
        ht = np.concatenate(segs, axis=1)
        m = dict(shared)
        m["hpa"] = hpa.reshape(128, -1)
        m["hpb"] = hpb.reshape(64, -1)
        m["ht"] = np.ascontiguousarray(ht.reshape(128, -1))
        in_maps.append(m)

    if "nc" not in _cached:
        _cached["nc"] = build_nc()
    res = run_bass_kernel_spmd(_cached["nc"], in_maps, core_ids=list(range(NCORES)),
                               trace=_trace)

    out = np.zeros((B, C, 2 * H, 2 * W), np.float32)
    for core in range(NCORES):
        b, half = core // 2, core % 2
        v = res.results[core]["out"].reshape(2, 64, 64, 256)
        out[b, :, half * 128:(half + 1) * 128, :] = \
            v.transpose(1, 2, 0, 3).reshape(64, 128, 256)
    if _trace:
        return out, res
    return out


# revision 25
# speedup vs baseline: 1.0509x; 1.0509x over previous
import sys

sys.path.insert(0, "/opt/trn_rl_repo")

import numpy as np
import ml_dtypes

import concourse.bacc as bacc
import concourse.bass as bass
import concourse.mybir as mybir
import concourse.tile as tile
from concourse.bass_utils import run_bass_kernel_spmd

F32 = mybir.dt.float32
F32R = mybir.dt.float32r
BF16 = mybir.dt.bfloat16
AF = mybir.ActivationFunctionType
ALU = mybir.AluOpType
AX = mybir.AxisListType

# Problem constants (hardcoded per harness contract).
B, C, H, W = 4, 64, 128, 128
NT = 9          # 3x3 taps
NFF = 4         # factor*factor subpixels
NCORES = 8
HL = H // 2     # 64 coarse rows per core
NYB = 4         # y-blocks
YB = HL // NYB  # 16 coarse rows per block
G = NFF * C * YB  # 4096 elements per tap-slice per partition

_cached = {}


def ap_of(t, off, dims):
    base = t[:]
    return bass.AP(base.tensor, base.offset + off, dims)


def build_nc():
    nc = bacc.Bacc("TRN2", target_bir_lowering=False, debug=False, num_devices=NCORES)

    hpa_d = nc.dram_tensor("hpa", [128, 66 * 128], BF16, kind="ExternalInput")
    hpb_d = nc.dram_tensor("hpb", [64, 66 * 128], BF16, kind="ExternalInput")
    ht_d = nc.dram_tensor("ht", [128, 4 * 9 * 64 * 16], BF16, kind="ExternalInput")
    w1a_d = nc.dram_tensor("w1a", [128, 3 * 128], BF16, kind="ExternalInput")
    w1b_d = nc.dram_tensor("w1b", [64, 3 * 128], BF16, kind="ExternalInput")
    b1_d = nc.dram_tensor("b1c", [128, 1], F32, kind="ExternalInput")
    w2t_d = nc.dram_tensor("w2t", [128, 36], BF16, kind="ExternalInput")
    b2_d = nc.dram_tensor("b2c", [36, 1], F32, kind="ExternalInput")
    idb_d = nc.dram_tensor("idb", [128, 128], BF16, kind="ExternalInput")
    out_d = nc.dram_tensor("out", [128, 64 * 256], F32, kind="ExternalOutput")

    with tile.TileContext(nc) as tc:
        with (
            tc.tile_pool(name="const", bufs=1) as cpool,
            tc.tile_pool(name="rin", bufs=2) as rin,
            tc.tile_pool(name="mchunk", bufs=3) as mpool,
            tc.tile_pool(name="rmask", bufs=2) as rmask,
            tc.tile_pool(name="work", bufs=1) as work,
            tc.tile_pool(name="racc", bufs=2) as racc,
            tc.tile_pool(name="orow", bufs=3) as opool,
            tc.tile_pool(name="ps1", bufs=2, space=bass.MemorySpace.PSUM) as pp1,
            tc.tile_pool(name="ps2", bufs=2, space=bass.MemorySpace.PSUM) as pp2,
            tc.tile_pool(name="pst", bufs=2, space=bass.MemorySpace.PSUM) as ppt,
            tc.tile_pool(name="pso", bufs=2, space=bass.MemorySpace.PSUM) as ppo,
        ):
            # ---- constants ----
            w1a = cpool.tile([128, 3 * 128], BF16)
            w1b = cpool.tile([64, 3 * 128], BF16)
            b1 = cpool.tile([128, 1], F32)
            w2t = cpool.tile([128, 36], BF16)
            b2 = cpool.tile([36, 1], F32)
            idb = cpool.tile([128, 128], BF16)
            nc.sync.dma_start(w1a[:], w1a_d[:])
            nc.sync.dma_start(w1b[:], w1b_d[:])
            nc.sync.dma_start(b1[:], b1_d[:])
            nc.sync.dma_start(w2t[:], w2t_d[:])
            nc.sync.dma_start(b2[:], b2_d[:])
            nc.sync.dma_start(idb[:], idb_d[:])

            BLOCKS = [(0, 8), (8, 16), (24, 16), (40, 16), (56, 8)]

            def phase_a(r0, nr):
                """DMA in, convs, mask, weighted tap sum -> acc (ffT, c, yl)."""
                nch = nr // 4
                gb = NFF * C * nr
                fcr = 64 * nr
                hto = 9 * 64 * r0
                hpab = rin.tile([128, 18 * 128], BF16, tag="hpab")
                hpbb = rin.tile([64, 18 * 128], BF16, tag="hpbb")
                htb = rin.tile([128, 9 * 64 * 16], BF16, tag="htb")
                nc.sync.dma_start(hpab[:, 0:(nr + 2) * 128],
                                  hpa_d[:, r0 * 128:(r0 + nr + 2) * 128])
                nc.sync.dma_start(hpbb[:, 0:(nr + 2) * 128],
                                  hpb_d[:, r0 * 128:(r0 + nr + 2) * 128])
                nc.sync.dma_start(htb[:, 0:9 * fcr],
                                  ht_d[:, hto:hto + 9 * fcr])

                eb = rmask.tile([36, 4 * 512], BF16, tag="eb")
                for ic in range(nch):
                    ps1 = pp1.tile([128, 512], F32)
                    for dy in range(3):
                        rhs = ap_of(hpab, (4 * ic + dy) * 128,
                                    [[18 * 128, 128], [1, 512]])
                        nc.tensor.matmul(ps1[:],
                                         w1a[:, dy * 128:(dy + 1) * 128],
                                         rhs, start=(dy == 0), stop=False)
                    for dy in range(3):
                        rhs = ap_of(hpbb, (4 * ic + dy) * 128,
                                    [[18 * 128, 64], [1, 512]])
                        nc.tensor.matmul(ps1[:],
                                         w1b[:, dy * 128:(dy + 1) * 128],
                                         rhs, start=False, stop=(dy == 2))
                    m = mpool.tile([128, 512], BF16)
                    nc.scalar.activation(m[:], ps1[:], AF.Relu, bias=b1[:], scale=1.0)
                    ps2 = pp2.tile([36, 512], F32)
                    nc.tensor.matmul(ps2[:], w2t[:], m[:])
                    nc.scalar.activation(eb[:, ic * 512:(ic + 1) * 512],
                                         ps2[:], AF.Exp, bias=b2[:], scale=1.0)

                # nmb layout: [x, (ff 4, t 9, yl nr)]; zt/rz: [x, (ff 4, yl nr)]
                zt = rmask.tile([128, 64], F32, tag="zt")
                rz = rmask.tile([128, 64], F32, tag="rz")
                nmb = rmask.tile([128, NFF * NT * YB], BF16, tag="nmb")
                for j in range(nch):
                    pst = ppt.tile([128, 4 * 36], BF16)
                    for r in range(4):
                        yl = 4 * j + r
                        nc.tensor.transpose(pst[:, r * 36:(r + 1) * 36],
                                            eb[:, yl * 128:(yl + 1) * 128],
                                            idb[0:36, 0:36])
                    nm_out = ap_of(nmb, 4 * j,
                                   [[NFF * NT * YB, 128], [NT * nr, 4], [nr, 9], [1, 4]])
                    nm_in = ap_of(pst, 0, [[4 * 36, 128], [9, 4], [1, 9], [36, 4]])
                    nc.scalar.copy(nm_out, nm_in)
                nc.vector.tensor_reduce(
                    ap_of(zt, 0, [[64, 128], [nr, 4], [1, nr]]),
                    ap_of(nmb, 0, [[NFF * NT * YB, 128], [9 * nr, 4], [1, nr], [nr, 9]]),
                    AX.X, ALU.add)
                nc.vector.reciprocal(rz[:, 0:4 * nr], zt[:, 0:4 * nr])
                nm2 = rmask.tile([128, NFF * NT * YB], BF16, tag="nm2")
                nc.vector.tensor_tensor(
                    ap_of(nm2, 0, [[NFF * NT * YB, 128], [9 * nr, 4], [nr, 9], [1, nr]]),
                    ap_of(nmb, 0, [[NFF * NT * YB, 128], [9 * nr, 4], [nr, 9], [1, nr]]),
                    ap_of(rz, 0, [[64, 128], [nr, 4], [0, 9], [1, nr]]),
                    ALU.mult)

                # weighted tap sum (TT 2x bf16), prod: [x, (t 9, ff 4, c 64, yl nr)]
                prod = work.tile([128, NT * G], BF16, tag="prod")
                tA = work.tile([128, 4 * G], BF16, tag="tA")
                acc = racc.tile([128, G], BF16, tag="acc")

                def pr(off, n):
                    return ap_of(prod, off * gb, [[NT * G, 128], [1, n * gb]])

                def ta(off, n):
                    return ap_of(tA, off * gb, [[4 * G, 128], [1, n * gb]])

                for t in range(9):
                    in0 = ap_of(htb, t * fcr,
                                [[9 * 64 * 16, 128], [0, 4], [nr, 64], [1, nr]])
                    in1 = ap_of(nm2, t * nr,
                                [[NFF * NT * YB, 128], [9 * nr, 4], [0, 64], [1, nr]])
                    po = ap_of(prod, t * gb,
                               [[NT * G, 128], [fcr, 4], [nr, 64], [1, nr]])
                    nc.vector.tensor_tensor(po, in0, in1, ALU.mult)
                nc.vector.tensor_tensor(tA[:, 0:4 * gb], pr(0, 4), pr(4, 4), ALU.add)
                nc.vector.tensor_tensor(pr(0, 2), ta(0, 2), ta(2, 2), ALU.add)
                nc.vector.tensor_tensor(ta(0, 1), pr(0, 1), pr(1, 1), ALU.add)
                nc.vector.tensor_tensor(acc[:, 0:gb], ta(0, 1), pr(8, 1), ALU.add)
                return acc, r0, nr

            def phase_b(ctx):
                """Pixel shuffle out: transpose (x,(fy,c)) -> ((fy,c),x), DMA."""
                acc, r0, nr = ctx
                fcr = 64 * nr
                for yp in range(nr // 4):
                    pso = ppo.tile([128, 1024], BF16)
                    for j4 in range(4):
                        yl = 4 * yp + j4
                        for fx in range(2):
                            in_ap = ap_of(acc, fx * 2 * fcr + yl,
                                          [[G, 128], [nr, 128]])
                            nc.tensor.transpose(
                                pso[:, (2 * j4 + fx) * 128:(2 * j4 + fx + 1) * 128],
                                in_ap, idb[:])
                    orow = opool.tile([128, 1024], F32)
                    co = ap_of(orow, 0, [[1024, 128], [256, 4], [1, 2], [2, 128]])
                    ci = ap_of(pso, 0, [[1024, 128], [256, 4], [128, 2], [1, 128]])
                    nc.scalar.copy(co, ci)
                    y = r0 + 4 * yp
                    nc.sync.dma_start(
                        ap_of(out_d, y * 256, [[64 * 256, 128], [1, 1024]]),
                        orow[:])

            pending = None
            for blk in BLOCKS:
                ctx = phase_a(*blk)
                if pending is not None:
                    phase_b(pending)
                pending = ctx
            phase_b(pending)

    nc.compile()
    return nc


def prep_shared(W1, b1, W2, b2):
    W1 = np.asarray(W1, np.float32)
    b1 = np.asarray(b1, np.float32)
    W2 = np.asarray(W2, np.float32).reshape(36, 128)
    b2 = np.asarray(b2, np.float32)

    w1a = np.zeros((128, 3 * 128), np.float32)
    w1b = np.zeros((64, 3 * 128), np.float32)
    for dy in range(3):
        w1a[0:64, dy * 128:(dy + 1) * 128] = W1[:, :, dy, 0].T
        w1a[64:128, dy * 128:(dy + 1) * 128] = W1[:, :, dy, 1].T
        w1b[:, dy * 128:(dy + 1) * 128] = W1[:, :, dy, 2].T

    # ffT = fx*2+fy ordering: mask channel for (ffT, t) is t*4 + (fy*2+fx)
    o_of_mp = np.array([t * 4 + 2 * (ffT % 2) + ffT // 2
                        for ffT in range(4) for t in range(9)])
    w2t = np.ascontiguousarray((0.25 * W2[o_of_mp, :]).T)
    b2c = np.ascontiguousarray((0.25 * b2[o_of_mp]).reshape(36, 1))

    return {
        "w1a": w1a.astype(ml_dtypes.bfloat16),
        "w1b": w1b.astype(ml_dtypes.bfloat16),
        "b1c": b1.reshape(128, 1).astype(np.float32),
        "w2t": w2t.astype(ml_dtypes.bfloat16), "b2c": b2c,
        "idb": np.eye(128, dtype=ml_dtypes.bfloat16),
    }


def kernel(h, W1, b1, W2, b2, _trace=False):
    h = np.asarray(h, np.float32)
    shared = prep_shared(W1, b1, W2, b2)

    hp = np.pad(h, ((0, 0), (0, 0), (1, 1), (1, 1)))  # [B, C, 130, 130]
    in_maps = []
    for core in range(NCORES):
        b, half = core // 2, core % 2
        y0 = half * HL
        win = hp[b, :, y0:y0 + 66, :]  # [64, 66, 130]
        winb = win.astype(ml_dtypes.bfloat16)
        hpa = np.empty((128, 66, 128), ml_dtypes.bfloat16)
        hpa[0:64] = winb[:, :, 0:128]
        hpa[64:128] = winb[:, :, 1:129]
        hpb = np.ascontiguousarray(winb[:, :, 2:130])
        win8 = (8.0 * win).astype(ml_dtypes.bfloat16)
        # ht: per-block segments [x, (t 9, c 64, yl nr)], blocks (0,8),(8,16)x3,(56,8)
        segs = []
        for r0, nr in [(0, 8), (8, 16), (24, 16), (40, 16), (56, 8)]:
            seg = np.empty((128, 9, 64, nr), ml_dtypes.bfloat16)
            for dy in range(3):
                for dx in range(3):
                    seg[:, dy * 3 + dx] = win8[
                        :, r0 + dy:r0 + dy + nr, dx:dx + 128].transpose(2, 0, 1)
            segs.append(seg.reshape(128, -1))
        ht = np.concatenate(segs, axis=1)
        m = dict(shared)
        m["hpa"] = hpa.reshape(128, -1)
        m["hpb"] = hpb.reshape(64, -1)
        m["ht"] = np.ascontiguousarray(ht.reshape(128, -1))
        in_maps.append(m)

    if "nc" not in _cached:
        _cached["nc"] = build_nc()
    res = run_bass_kernel_spmd(_cached["nc"], in_maps, core_ids=list(range(NCORES)),
                               trace=_trace)

    out = np.zeros((B, C, 2 * H, 2 * W), np.float32)
    for core in range(NCORES):
        b, half = core // 2, core % 2
        v = res.results[core]["out"].reshape(2, 64, 64, 256)
        out[b, :, half * 128:(half + 1) * 128, :] = \
            v.transpose(1, 2, 0, 3).reshape(64, 128, 256)
    if _trace:
        return out, res
    return out


# revision 28
# speedup vs baseline: 1.0515x; 1.0006x over previous
import sys

sys.path.insert(0, "/opt/trn_rl_repo")

import numpy as np
import ml_dtypes

import concourse.bacc as bacc
import concourse.bass as bass
import concourse.mybir as mybir
import concourse.tile as tile
from concourse.bass_utils import run_bass_kernel_spmd

F32 = mybir.dt.float32
F32R = mybir.dt.float32r
BF16 = mybir.dt.bfloat16
AF = mybir.ActivationFunctionType
ALU = mybir.AluOpType
AX = mybir.AxisListType

# Problem constants (hardcoded per harness contract).
B, C, H, W = 4, 64, 128, 128
NT = 9          # 3x3 taps
NFF = 4         # factor*factor subpixels
NCORES = 8
HL = H // 2     # 64 coarse rows per core
NYB = 4         # y-blocks
YB = HL // NYB  # 16 coarse rows per block
G = NFF * C * YB  # 4096 elements per tap-slice per partition
BLOCKS = [(0, 4), (4, 8), (12, 16), (28, 16), (44, 16), (60, 4)]

_cached = {}


def ap_of(t, off, dims):
    base = t[:]
    return bass.AP(base.tensor, base.offset + off, dims)


def build_nc():
    nc = bacc.Bacc("TRN2", target_bir_lowering=False, debug=False, num_devices=NCORES)

    hpa_d = nc.dram_tensor("hpa", [128, 66 * 128], BF16, kind="ExternalInput")
    hpb_d = nc.dram_tensor("hpb", [64, 66 * 128], BF16, kind="ExternalInput")
    ht_d = nc.dram_tensor("ht", [128, 4 * 9 * 64 * 16], BF16, kind="ExternalInput")
    w1a_d = nc.dram_tensor("w1a", [128, 3 * 128], BF16, kind="ExternalInput")
    w1b_d = nc.dram_tensor("w1b", [64, 3 * 128], BF16, kind="ExternalInput")
    b1_d = nc.dram_tensor("b1c", [128, 1], F32, kind="ExternalInput")
    w2t_d = nc.dram_tensor("w2t", [128, 36], BF16, kind="ExternalInput")
    b2_d = nc.dram_tensor("b2c", [36, 1], F32, kind="ExternalInput")
    idb_d = nc.dram_tensor("idb", [128, 128], BF16, kind="ExternalInput")
    out_d = nc.dram_tensor("out", [128, 64 * 256], F32, kind="ExternalOutput")

    with tile.TileContext(nc) as tc:
        with (
            tc.tile_pool(name="const", bufs=1) as cpool,
            tc.tile_pool(name="rin", bufs=2) as rin,
            tc.tile_pool(name="mchunk", bufs=3) as mpool,
            tc.tile_pool(name="rmask", bufs=2) as rmask,
            tc.tile_pool(name="work", bufs=1) as work,
            tc.tile_pool(name="racc", bufs=2) as racc,
            tc.tile_pool(name="orow", bufs=3) as opool,
            tc.tile_pool(name="ps1", bufs=2, space=bass.MemorySpace.PSUM) as pp1,
            tc.tile_pool(name="ps2", bufs=2, space=bass.MemorySpace.PSUM) as pp2,
            tc.tile_pool(name="pst", bufs=2, space=bass.MemorySpace.PSUM) as ppt,
            tc.tile_pool(name="pso", bufs=2, space=bass.MemorySpace.PSUM) as ppo,
        ):
            # ---- constants ----
            w1a = cpool.tile([128, 3 * 128], BF16)
            w1b = cpool.tile([64, 3 * 128], BF16)
            b1 = cpool.tile([128, 1], F32)
            w2t = cpool.tile([128, 36], BF16)
            b2 = cpool.tile([36, 1], F32)
            idb = cpool.tile([128, 128], BF16)
            nc.sync.dma_start(w1a[:], w1a_d[:])
            nc.sync.dma_start(w1b[:], w1b_d[:])
            nc.sync.dma_start(b1[:], b1_d[:])
            nc.sync.dma_start(w2t[:], w2t_d[:])
            nc.sync.dma_start(b2[:], b2_d[:])
            nc.sync.dma_start(idb[:], idb_d[:])


            def phase_a(r0, nr):
                """DMA in, convs, mask, weighted tap sum -> acc (ffT, c, yl)."""
                nch = nr // 4
                gb = NFF * C * nr
                fcr = 64 * nr
                hto = 9 * 64 * r0
                hpab = rin.tile([128, 18 * 128], BF16, tag="hpab")
                hpbb = rin.tile([64, 18 * 128], BF16, tag="hpbb")
                htb = rin.tile([128, 9 * 64 * 16], BF16, tag="htb")
                nc.sync.dma_start(hpab[:, 0:(nr + 2) * 128],
                                  hpa_d[:, r0 * 128:(r0 + nr + 2) * 128])
                nc.sync.dma_start(hpbb[:, 0:(nr + 2) * 128],
                                  hpb_d[:, r0 * 128:(r0 + nr + 2) * 128])
                nc.sync.dma_start(htb[:, 0:9 * fcr],
                                  ht_d[:, hto:hto + 9 * fcr])

                eb = rmask.tile([36, 4 * 512], BF16, tag="eb")
                for ic in range(nch):
                    ps1 = pp1.tile([128, 512], F32)
                    for dy in range(3):
                        rhs = ap_of(hpab, (4 * ic + dy) * 128,
                                    [[18 * 128, 128], [1, 512]])
                        nc.tensor.matmul(ps1[:],
                                         w1a[:, dy * 128:(dy + 1) * 128],
                                         rhs, start=(dy == 0), stop=False)
                    for dy in range(3):
                        rhs = ap_of(hpbb, (4 * ic + dy) * 128,
                                    [[18 * 128, 64], [1, 512]])
                        nc.tensor.matmul(ps1[:],
                                         w1b[:, dy * 128:(dy + 1) * 128],
                                         rhs, start=False, stop=(dy == 2))
                    m = mpool.tile([128, 512], BF16)
                    nc.scalar.activation(m[:], ps1[:], AF.Relu, bias=b1[:], scale=1.0)
                    ps2 = pp2.tile([36, 512], F32)
                    nc.tensor.matmul(ps2[:], w2t[:], m[:])
                    nc.scalar.activation(eb[:, ic * 512:(ic + 1) * 512],
                                         ps2[:], AF.Exp, bias=b2[:], scale=1.0)

                # nmb layout: [x, (ff 4, t 9, yl nr)]; zt/rz: [x, (ff 4, yl nr)]
                zt = rmask.tile([128, 64], F32, tag="zt")
                rz = rmask.tile([128, 64], F32, tag="rz")
                nmb = rmask.tile([128, NFF * NT * YB], BF16, tag="nmb")
                for j in range(nch):
                    pst = ppt.tile([128, 4 * 36], BF16)
                    for r in range(4):
                        yl = 4 * j + r
                        nc.tensor.transpose(pst[:, r * 36:(r + 1) * 36],
                                            eb[:, yl * 128:(yl + 1) * 128],
                                            idb[0:36, 0:36])
                    nm_out = ap_of(nmb, 4 * j,
                                   [[NFF * NT * YB, 128], [NT * nr, 4], [nr, 9], [1, 4]])
                    nm_in = ap_of(pst, 0, [[4 * 36, 128], [9, 4], [1, 9], [36, 4]])
                    nc.scalar.copy(nm_out, nm_in)
                nc.vector.tensor_reduce(
                    ap_of(zt, 0, [[64, 128], [nr, 4], [1, nr]]),
                    ap_of(nmb, 0, [[NFF * NT * YB, 128], [9 * nr, 4], [1, nr], [nr, 9]]),
                    AX.X, ALU.add)
                nc.vector.reciprocal(rz[:, 0:4 * nr], zt[:, 0:4 * nr])
                nm2 = rmask.tile([128, NFF * NT * YB], BF16, tag="nm2")
                nc.vector.tensor_tensor(
                    ap_of(nm2, 0, [[NFF * NT * YB, 128], [9 * nr, 4], [nr, 9], [1, nr]]),
                    ap_of(nmb, 0, [[NFF * NT * YB, 128], [9 * nr, 4], [nr, 9], [1, nr]]),
                    ap_of(rz, 0, [[64, 128], [nr, 4], [0, 9], [1, nr]]),
                    ALU.mult)

                # weighted tap sum (TT 2x bf16), prod: [x, (t 9, ff 4, c 64, yl nr)]
                prod = work.tile([128, NT * G], BF16, tag="prod")
                tA = work.tile([128, 4 * G], BF16, tag="tA")
                acc = racc.tile([128, G], BF16, tag="acc")

                def pr(off, n):
                    return ap_of(prod, off * gb, [[NT * G, 128], [1, n * gb]])

                def ta(off, n):
                    return ap_of(tA, off * gb, [[4 * G, 128], [1, n * gb]])

                for t in range(9):
                    in0 = ap_of(htb, t * fcr,
                                [[9 * 64 * 16, 128], [0, 4], [nr, 64], [1, nr]])
                    in1 = ap_of(nm2, t * nr,
                                [[NFF * NT * YB, 128], [9 * nr, 4], [0, 64], [1, nr]])
                    po = ap_of(prod, t * gb,
                               [[NT * G, 128], [fcr, 4], [nr, 64], [1, nr]])
                    nc.vector.tensor_tensor(po, in0, in1, ALU.mult)
                nc.vector.tensor_tensor(tA[:, 0:4 * gb], pr(0, 4), pr(4, 4), ALU.add)
                nc.vector.tensor_tensor(pr(0, 2), ta(0, 2), ta(2, 2), ALU.add)
                nc.vector.tensor_tensor(ta(0, 1), pr(0, 1), pr(1, 1), ALU.add)
                nc.vector.tensor_tensor(acc[:, 0:gb], ta(0, 1), pr(8, 1), ALU.add)
                return acc, r0, nr

            def phase_b(ctx):
                """Pixel shuffle out: transpose (x,(fy,c)) -> ((fy,c),x), DMA."""
                acc, r0, nr = ctx
                fcr = 64 * nr
                for yp in range(nr // 4):
                    pso = ppo.tile([128, 1024], BF16)
                    for j4 in range(4):
                        yl = 4 * yp + j4
                        for fx in range(2):
                            in_ap = ap_of(acc, fx * 2 * fcr + yl,
                                          [[G, 128], [nr, 128]])
                            nc.tensor.transpose(
                                pso[:, (2 * j4 + fx) * 128:(2 * j4 + fx + 1) * 128],
                                in_ap, idb[:])
                    orow = opool.tile([128, 1024], F32)
                    co = ap_of(orow, 0, [[1024, 128], [256, 4], [1, 2], [2, 128]])
                    ci = ap_of(pso, 0, [[1024, 128], [256, 4], [128, 2], [1, 128]])
                    nc.scalar.copy(co, ci)
                    y = r0 + 4 * yp
                    nc.sync.dma_start(
                        ap_of(out_d, y * 256, [[64 * 256, 128], [1, 1024]]),
                        orow[:])

            pending = None
            for blk in BLOCKS:
                ctx = phase_a(*blk)
                if pending is not None:
                    phase_b(pending)
                pending = ctx
            phase_b(pending)

    nc.compile()
    return nc


def prep_shared(W1, b1, W2, b2):
    W1 = np.asarray(W1, np.float32)
    b1 = np.asarray(b1, np.float32)
    W2 = np.asarray(W2, np.float32).reshape(36, 128)
    b2 = np.asarray(b2, np.float32)

    w1a = np.zeros((128, 3 * 128), np.float32)
    w1b = np.zeros((64, 3 * 128), np.float32)
    for dy in range(3):
        w1a[0:64, dy * 128:(dy + 1) * 128] = W1[:, :, dy, 0].T
        w1a[64:128, dy * 128:(dy + 1) * 128] = W1[:, :, dy, 1].T
        w1b[:, dy * 128:(dy + 1) * 128] = W1[:, :, dy, 2].T

    # ffT = fx*2+fy ordering: mask channel for (ffT, t) is t*4 + (fy*2+fx)
    o_of_mp = np.array([t * 4 + 2 * (ffT % 2) + ffT // 2
                        for ffT in range(4) for t in range(9)])
    w2t = np.ascontiguousarray((0.25 * W2[o_of_mp, :]).T)
    b2c = np.ascontiguousarray((0.25 * b2[o_of_mp]).reshape(36, 1))

    return {
        "w1a": w1a.astype(ml_dtypes.bfloat16),
        "w1b": w1b.astype(ml_dtypes.bfloat16),
        "b1c": b1.reshape(128, 1).astype(np.float32),
        "w2t": w2t.astype(ml_dtypes.bfloat16), "b2c": b2c,
        "idb": np.eye(128, dtype=ml_dtypes.bfloat16),
    }


def kernel(h, W1, b1, W2, b2, _trace=False):
    h = np.asarray(h, np.float32)
    shared = prep_shared(W1, b1, W2, b2)

    hp = np.pad(h, ((0, 0), (0, 0), (1, 1), (1, 1)))  # [B, C, 130, 130]
    in_maps = []
    for core in range(NCORES):
        b, half = core // 2, core % 2
        y0 = half * HL
        win = hp[b, :, y0:y0 + 66, :]  # [64, 66, 130]
        winb = win.astype(ml_dtypes.bfloat16)
        hpa = np.empty((128, 66, 128), ml_dtypes.bfloat16)
        hpa[0:64] = winb[:, :, 0:128]
        hpa[64:128] = winb[:, :, 1:129]
        hpb = np.ascontiguousarray(winb[:, :, 2:130])
        win8 = (8.0 * win).astype(ml_dtypes.bfloat16)
        # ht: per-block segments [x, (t 9, c 64, yl nr)], blocks (0,8),(8,16)x3,(56,8)
        segs = []
        for r0, nr in BLOCKS:
            seg = np.empty((128, 9, 64, nr), ml_dtypes.bfloat16)
            for dy in range(3):
                for dx in range(3):
                    seg[:, dy * 3 + dx] = win8[
                        :, r0 + dy:r0 + dy + nr, dx:dx + 128].transpose(2, 0, 1)
            segs.append(seg.reshape(128, -1))
        ht = np.concatenate(segs, axis=1)
        m = dict(shared)
        m["hpa"] = hpa.reshape(128, -1)
        m["hpb"] = hpb.reshape(64, -1)
        m["ht"] = np.ascontiguousarray(ht.reshape(128, -1))
        in_maps.append(m)

    if "nc" not in _cached:
        _cached["nc"] = build_nc()
    res = run_bass_kernel_spmd(_cached["nc"], in_maps, core_ids=list(range(NCORES)),
                               trace=_trace)

    out = np.zeros((B, C, 2 * H, 2 * W), np.float32)
    for core in range(NCORES):
        b, half = core // 2, core % 2
        v = res.results[core]["out"].reshape(2, 64, 64, 256)
        out[b, :, half * 128:(half + 1) * 128, :] = \
            v.transpose(1, 2, 0, 3).reshape(64, 128, 256)
    if _trace:
        return out, res
    return out
